# revision 1
# baseline (speedup 1.0000x reference)
"""Trainium2 Bass kernel for BHS_GCN: 2x GCNConv + dueling value/advantage heads.

Strategy (8 NeuronCores, single NEFF launch):
  - GCN phase batch-parallel: each core owns B_LOC=4 full graphs.
    Message passing = bulk dma_gather of source-node rows + PE one-hot
    scatter-matmuls into PSUM (edges pre-sorted/packed by dst on host).
  - AllToAll reshards the pre-W2 aggregation (agg2, [N,4,128] per core) to
    node-parallel: each core gets its 512-node slice for all 32 batches.
  - W2 matmul + relu + head contraction run node-sharded, so each core reads
    only its 1/8 slice of advW/v1W (the 318MB dominating memory traffic is
    read once machine-wide).
  - AllReduce of [76,32] partial head sums; the tiny val-MLP and dueling
    combine run redundantly on every core; host takes core 0's output.
"""

import sys

sys.path.insert(0, "/opt/trn_rl_repo")

import os

import numpy as np
import ml_dtypes

# Precision mode: "f32" (exact), "bf16" (everything big in bf16), or a
# comma-set of {mp2,xfer,head}: mp2 = H1/messages/one-hots; xfer = A2A
# payload + W2; head = H2 + head weights. Accumulation is always fp32 PSUM.
PRECISION = os.environ.get("GCN_PREC", "f32")
BF16 = np.dtype(ml_dtypes.bfloat16)


def _prec_groups():
    if PRECISION == "f32":
        return set()
    if PRECISION == "bf16":
        return {"mp2", "xfer", "head"}
    return set(PRECISION.split(","))


PREC_G = _prec_groups()

# ---------------- problem constants (hardcoded per contract) ----------------
B, N, F_IN, E = 32, 4096, 16, 16384
NC_CORES = 8
B_LOC = B // NC_CORES            # 4
NSLICE = N // NC_CORES           # 512 nodes per core for head phase
F1, F2 = 128, 256
P = 128
NTILES = N // P                  # 32 node tiles
BF1 = B_LOC * F_IN               # 64   (mp1 row width)
BFH = B_LOC * F1                 # 512  (H1 row width = mp2 gather width)
KTOT = NSLICE * F2               # 131072 contraction rows per core
KT = KTOT // P                   # 1024 K-tiles for head matmul
HW_W = 12 + 64                   # 76 head outputs (adv | v1)
# bf16 mode pads head-weight tiles to 128 cols so the PE fast-weight-load
# (FWL, needs NumWeights==128 and non-fp32) engages: 4x faster ldweights.
HW_P = 128 if "head" in PREC_G else HW_W
NT_HEAD = 16                     # nodes per W2/head block


def _pack_edges(edge_index, edge_weight):
    """Sort edges (+ self loops) by dst, pack into 128-edge chunks such that
    every chunk's dsts fall in one 128-node tile. Returns device tables."""
    src = np.asarray(edge_index[0], np.int64)
    dst = np.asarray(edge_index[1], np.int64)
    ew = np.asarray(edge_weight, np.float32)

    deg = np.zeros(N, np.float32)
    np.add.at(deg, dst, ew)
    deg += 1.0
    dinv = (1.0 / np.sqrt(deg)).astype(np.float32)

    norm = ew * dinv[src] * dinv[dst]
    # self loops: src=dst=n, weight 1/deg[n]
    src_a = np.concatenate([src, np.arange(N, dtype=np.int64)])
    dst_a = np.concatenate([dst, np.arange(N, dtype=np.int64)])
    nrm_a = np.concatenate([norm, dinv * dinv]).astype(np.float32)

    order = np.argsort(dst_a, kind="stable")
    src_a, dst_a, nrm_a = src_a[order], dst_a[order], nrm_a[order]

    src_pk, nrm_pk, off_pk = [], [], []
    chunk_tile = []
    for t in range(NTILES):
        sel = (dst_a >= t * P) & (dst_a < (t + 1) * P)
        s, d, w = src_a[sel], dst_a[sel], nrm_a[sel]
        cnt = len(s)
        nch = max(1, (cnt + P - 1) // P)
        pad = nch * P - cnt
        src_pk.append(np.concatenate([s, np.zeros(pad, np.int64)]))
        nrm_pk.append(np.concatenate([w, np.zeros(pad, np.float32)]))
        off_pk.append(np.concatenate([d - t * P, np.zeros(pad, np.int64)]))
        chunk_tile.extend([t] * nch)

    src_pk = np.concatenate(src_pk)
    nrm_pk = np.concatenate(nrm_pk)
    off_pk = np.concatenate(off_pk)
    e_pad = len(src_pk)
    nchunk = e_pad // P
    assert nchunk == len(chunk_tile)

    # dma_gather index table: logical idx i lives at [i % 16, i // 16]
    gidx = np.zeros((P, e_pad // 16), np.int16)
    for p16 in range(16):
        gidx[p16, :] = src_pk[p16::16].astype(np.int16)
    gidx = np.tile(gidx[:16], (8, 1))  # replicate over all 128 partitions

    # per-chunk column tables: [p, c] = value of edge c*128+p
    nrm_t = nrm_pk.reshape(nchunk, P).T.copy()          # [128, nchunk] f32
    off_t = off_pk.reshape(nchunk, P).T.astype(np.float32).copy()
    return gidx, nrm_t, off_t, chunk_tile, nchunk


def _prep_host(inputs):
    """All host-side numpy preprocessing: edge packing, weight layout, batch shard."""
    x = np.asarray(inputs["x"], np.float32)
    gidx, nrm_t, off_t, chunk_tile, nchunk = _pack_edges(
        inputs["edge_index"], inputs["edge_weight"]
    )

    W1 = np.asarray(inputs["W1"], np.float32)      # [16,128]
    b1 = np.asarray(inputs["b1"], np.float32)      # [128]
    W2 = np.asarray(inputs["W2"], np.float32)      # [128,256]
    b2 = np.asarray(inputs["b2"], np.float32)      # [256]
    advW = np.asarray(inputs["advW"], np.float32)  # [N*256, 12]
    advb = np.asarray(inputs["advb"], np.float32)
    v1W = np.asarray(inputs["v1W"], np.float32)    # [N*256, 64]
    v1b = np.asarray(inputs["v1b"], np.float32)
    v2W = np.asarray(inputs["v2W"], np.float32)
    v2b = np.asarray(inputs["v2b"], np.float32)
    v3W = np.asarray(inputs["v3W"], np.float32)
    v3b = np.asarray(inputs["v3b"], np.float32)

    # W1 block-diagonal over the 4 local batches, plus a bias row driven by
    # a constant-1 row appended to aggT on device: [65, 512]
    w1bd = np.zeros((BF1 + 1, B_LOC * F1), np.float32)
    for b in range(B_LOC):
        w1bd[b * F_IN:(b + 1) * F_IN, b * F1:(b + 1) * F1] = W1
    w1bd[BF1, :] = np.tile(b1, B_LOC)

    # dueling combine matrix (adv part): out = C.T @ adv + val
    C = np.zeros((12, 12), np.float32)
    for h in range(3):
        for a in range(4):
            i = h * 4 + a
            C[i, i] += 1.0
            for a2 in range(4):
                C[h * 4 + a2, i] -= 0.25

    shared = {
        "gidx": gidx,
        "nrm_t": nrm_t.copy(),
        "off_t": off_t.copy(),
        "w1bd": w1bd,
        "w2": (W2.astype(BF16) if "xfer" in PREC_G else W2).copy(),
        "b2c": b2[:, None].copy(),                  # [256,1]
        "advb_c": advb[:, None].copy(),             # [12,1]
        "v1b_c": v1b[:, None].copy(),               # [64,1]
        "v2w": v2W.copy(),                          # [64,64]
        "v2b_c": v2b[:, None].copy(),               # [64,1]
        "v3w": v3W.copy(),                          # [64,1]
        "v3b_c": v3b[None, :].copy(),               # [1,1]
        "cmat": C,
    }

    per_core = []
    for j in range(NC_CORES):
        # x batch-shard, node-major rows [N, b, f] -> [N, 64]
        x_loc = x[j * B_LOC:(j + 1) * B_LOC].transpose(1, 0, 2).reshape(N, BF1).copy()
        # head weights: rows for this core's node slice, pre-tiled to
        # [128, KT*76]: col block j holds lhsT K-tile j = rows [128j,128j+128)
        r0 = j * KTOT
        aw = advW[r0:r0 + KTOT].reshape(KT, P, 12)
        vw = v1W[r0:r0 + KTOT].reshape(KT, P, 64)
        # v1 first (partitions 0:64), adv second (64:76): partition slices
        # must start at multiples of 32 on-device.
        pad = np.zeros((KT, P, HW_P - HW_W), np.float32)
        hw = np.concatenate([vw, aw, pad], axis=2)  # [KT, 128, HW_P]
        hw_t = hw.transpose(1, 0, 2).reshape(P, KT * HW_P)
        hw_t = (hw_t.astype(BF16) if "head" in PREC_G else hw_t).copy()
        per_core.append({"x_loc": x_loc, "headw_t": hw_t})

    return shared, per_core, chunk_tile, nchunk


# ---------------- device program ----------------

def build_program(nc, tc, chunk_tile, nchunk, io, collectives=True, phases=(1,1,1), repeat=1):
    """Emit the Tile program. io: dict of name -> DRAM AP."""
    import concourse.bass as bass
    import concourse.mybir as mybir
    import concourse.tile as tile
    from concourse.masks import make_identity

    f32 = mybir.dt.float32
    bf16 = mybir.dt.bfloat16
    mp2_dt = bf16 if "mp2" in PREC_G else f32
    xf_dt = bf16 if "xfer" in PREC_G else f32
    hd_dt = bf16 if "head" in PREC_G else f32
    i16 = mybir.dt.int16
    i32 = mybir.dt.int32
    AF = mybir.ActivationFunctionType
    OP = mybir.AluOpType

    e_pad = nchunk * P
    # chunks belonging to each node tile (contiguous ranges)
    tile_chunks = [[] for _ in range(NTILES)]
    for c, t in enumerate(chunk_tile):
        tile_chunks[t].append(c)

    from contextlib import ExitStack
    with ExitStack() as ctx:
        const = ctx.enter_context(tc.tile_pool(name="const", bufs=1))
        sb = ctx.enter_context(tc.tile_pool(name="sb", bufs=3))
        sb_msg = ctx.enter_context(tc.tile_pool(name="msg", bufs=3))
        sb_s = ctx.enter_context(tc.tile_pool(name="sbs", bufs=3))
        sb_hw = ctx.enter_context(tc.tile_pool(name="sbhw", bufs=2))
        ps_agg = ctx.enter_context(tc.tile_pool(name="ps_agg", bufs=2, space="PSUM"))
        ps_t = ctx.enter_context(tc.tile_pool(name="ps_t", bufs=4, space="PSUM"))
        ps_head = ctx.enter_context(tc.tile_pool(name="ps_head", bufs=1, space="PSUM"))
        dram = ctx.enter_context(tc.tile_pool(name="dram", bufs=1, space="DRAM"))
        for _rep in range(repeat):
            # ---- constants into SBUF
            ident = const.tile([P, P], f32)
            make_identity(nc, ident[:])
            iota_i = const.tile([P, P], i32)
            nc.gpsimd.iota(iota_i[:], pattern=[[1, P]], base=0, channel_multiplier=0)
            iota_f = const.tile([P, P], f32)
            nc.vector.tensor_copy(iota_f[:], iota_i[:])
            ones1 = const.tile([1, P], f32)
            nc.vector.memset(ones1[:], 1.0)

            gidx_sb = const.tile([P, e_pad // 16], i16)
            nc.sync.dma_start(gidx_sb[:], io["gidx"][:, :])
            nrm_sb = const.tile([P, nchunk], f32)
            nc.sync.dma_start(nrm_sb[:], io["nrm_t"][:, :])
            off_sb = const.tile([P, nchunk], f32)
            nc.sync.dma_start(off_sb[:], io["off_t"][:, :])

            w1bd_sb = const.tile([BF1 + 1, B_LOC * F1], f32)
            nc.sync.dma_start(w1bd_sb[:], io["w1bd"][:, :])
            w2_sb = const.tile([P, F2], xf_dt)
            nc.sync.dma_start(w2_sb[:], io["w2"][:, :])
            # b2 [256,1] -> two [128,1] sbuf column stacks
            b2a = const.tile([P, 2], f32)
            nc.sync.dma_start(b2a[:, 0:1], io["b2c"][0:P, :])
            nc.sync.dma_start(b2a[:, 1:2], io["b2c"][P:F2, :])
            advb_sb = const.tile([12, 1], f32)
            nc.sync.dma_start(advb_sb[:], io["advb_c"][:, :])
            v1b_sb = const.tile([64, 1], f32)
            nc.sync.dma_start(v1b_sb[:], io["v1b_c"][:, :])
            v2w_sb = const.tile([64, 64], f32)
            nc.sync.dma_start(v2w_sb[:], io["v2w"][:, :])
            v2b_sb = const.tile([64, 1], f32)
            nc.sync.dma_start(v2b_sb[:], io["v2b_c"][:, :])
            v3w_sb = const.tile([64, 1], f32)
            nc.sync.dma_start(v3w_sb[:], io["v3w"][:, :])
            v3b_sb = const.tile([1, 1], f32)
            nc.sync.dma_start(v3b_sb[:], io["v3b_c"][:, :])
            cmat_sb = const.tile([12, 12], f32)
            nc.sync.dma_start(cmat_sb[:], io["cmat"][:, :])

            # scratch DRAM
            h1_dram = dram.tile([N, BFH], mp2_dt)        # node-major H1
            # agg2 feature-major, one buffer pair per node-quarter so the
            # AllToAll pipelines with mp2 and the head phase:
            # a2a_*_q[q][k, fin, n128, b]
            a2a_in_q = [dram.tile([NC_CORES, F1, P, B_LOC], xf_dt,
                                  name=f"a2ain{q}") for q in range(4)]
            a2a_out_q = [dram.tile([NC_CORES, F1, P, B_LOC], xf_dt,
                                   name=f"a2aout{q}") for q in range(4)]
            ar_in = dram.tile([HW_W, B], f32)
            ar_out = dram.tile([HW_W, B], f32)

            # ================= mp1 + L1 feature matmul =================
            def mp_layer(x_dram, elem, out_cb, dt=f32, order=None):
                """gather + scatter for one GCN layer; out_cb(t, agg_psum_ap).

                One-hot scatter matrices S[c][e, n] = norm[e] * (dstoff[e] == n)
                are built per node tile on DVE (rebuilt per layer to save SBUF)."""
                for t in (order if order is not None else range(NTILES)):
                    cs = tile_chunks[t]
                    c0, nch = cs[0], len(cs)
                    msg = sb_msg.tile([P, nch * elem], dt, tag="msg")
                    nidx = nch * P
                    nc.gpsimd.dma_gather(
                        out_ap=msg[:].rearrange("p (c e) -> p c e", e=elem),
                        in_ap=x_dram[:, :],
                        idxs_ap=gidx_sb[:, c0 * 8:(c0 + nch) * 8],
                        num_idxs=nidx,
                        num_idxs_reg=nidx,
                        elem_size=elem,
                    )
                    s_t = sb_s.tile([P, nch * P], dt, tag="sC")
                    for i, c in enumerate(cs):
                        # S[e, n] = (iota[n] == dstoff[e]) * norm[e], one fused op
                        nc.vector.tensor_scalar(
                            out=s_t[:, i * P:(i + 1) * P], in0=iota_f[:],
                            scalar1=off_sb[:, c:c + 1], scalar2=nrm_sb[:, c:c + 1],
                            op0=OP.is_equal, op1=OP.mult,
                        )
                    agg = ps_agg.tile([P, elem], f32, tag="agg")
                    for i, c in enumerate(cs):
                        nc.tensor.matmul(
                            agg[:],
                            lhsT=s_t[:, i * P:(i + 1) * P],
                            rhs=msg[:, i * elem:(i + 1) * elem],
                            start=(i == 0),
                            stop=(i == nch - 1),
                        )
                    out_cb(t, agg)

            # ---- layer 1
            def l1_out(t, agg):
                # transpose agg [128n, 64] -> aggT [64, 128n]
                agg_sb = sb.tile([P, BF1], f32, tag="agg1sb")
                nc.vector.tensor_copy(agg_sb[:], agg[:])
                psT = ps_t.tile([BF1, P], f32, tag="work")
                nc.tensor.transpose(psT[:], agg_sb[:], ident[:])
                aggT = sb.tile([BF1 + 1, P], f32, tag="aggT1")
                nc.vector.tensor_copy(aggT[0:BF1, :], psT[:])
                nc.vector.memset(aggT[BF1:BF1 + 1, :], 1.0)
                # H1[t] = relu(aggT_aug.T @ w1bd_aug)  (last row carries b1)
                psH = ps_t.tile([P, B_LOC * F1], f32, tag="work")
                nc.tensor.matmul(psH[:], lhsT=aggT[:], rhs=w1bd_sb[:], start=True, stop=True)
                h1sb = sb.tile([P, BFH], mp2_dt, tag="h1sb")
                nc.scalar.activation(h1sb[:], psH[:], AF.Relu)
                nc.sync.dma_start(h1_dram[t * P:(t + 1) * P, :], h1sb[:])

            if phases[0]:
                mp_layer(io["x_loc"], BF1, l1_out)

            # ---- layer 2 message passing -> a2a_in (feature-major [k, fin, n', b]
            # so the post-A2A W2-rhs reads are contiguous per partition)
            def l2_out(t, agg):
                agg_sb = sb.tile([P, BFH], f32, tag="agg2sb")
                nc.vector.tensor_copy(agg_sb[:], agg[:])
                a2a_sb = sb.tile([P, BFH], xf_dt, tag="a2asb")
                a2a_3d = a2a_sb[:].rearrange("f (n b) -> f n b", b=B_LOC)
                for b in range(B_LOC):
                    psT = ps_t.tile([P, P], f32, tag="work")
                    nc.tensor.transpose(psT[:], agg_sb[:, b * F1:(b + 1) * F1], ident[:])
                    nc.vector.tensor_copy(a2a_3d[:, :, b], psT[:])
                k, q = t // 4, t % 4
                nc.sync.dma_start(a2a_in_q[q][k, :, :, :], a2a_sb[:])

            if phases[1]:
                # quarter-major order: all of quarter q's tiles finish together,
                # releasing A2A chunk q while mp2 continues on quarter q+1
                mp_layer(h1_dram, BFH, l2_out, dt=mp2_dt,
                         order=[4 * k + q for q in range(4) for k in range(8)])

            # ====== per-quarter AllToAll chunk + W2 + head partials ======
            NB_H = 64
            SB_COLS = NB_H * B_LOC                       # 256 cols per src core
            ps_hd = ps_head.tile([HW_P, B], f32)
            nblocks = (NSLICE // NB_H) if phases[2] else 0
            first_mm = True
            if not phases[2]:
                nc.vector.memset(ps_hd[:], 0.0)

            def a2a_chunk(q):
                if collectives:
                    nc.gpsimd.collective_compute(
                        "AllToAll",
                        mybir.AluOpType.bypass,
                        replica_groups=[list(range(NC_CORES))],
                        ins=[a2a_in_q[q][:].opt()],
                        outs=[a2a_out_q[q][:].opt()],
                    )
                else:
                    for s in range(NC_CORES):
                        nc.sync.dma_start(
                            a2a_out_q[q][s].rearrange("f n b -> f (n b)"),
                            a2a_in_q[q][s].rearrange("f n b -> f (n b)"),
                        )

            for nb in range(nblocks):
                q, half = nb // 2, nb % 2
                if half == 0:
                    a2a_chunk(q)
                # stage rhs [128 fin, (s, n, b)]: contiguous 1KB runs per partition
                rhs_sb = sb.tile([P, NC_CORES * SB_COLS], xf_dt, tag="w2rhs")
                for s in range(NC_CORES):
                    nc.sync.dma_start(
                        rhs_sb[:, s * SB_COLS:(s + 1) * SB_COLS],
                        a2a_out_q[q][s, :, half * NB_H:(half + 1) * NB_H, :],
                    )
                h2 = []
                for fh in range(2):
                    h2sb = sb.tile([P, NC_CORES * SB_COLS], hd_dt, tag="h2sb")
                    for q in range(4):  # free split: 512-col matmuls
                        sl = slice(q * 512, (q + 1) * 512)
                        psW = ps_t.tile([P, 512], f32, tag="work")
                        nc.tensor.matmul(
                            psW[:], lhsT=w2_sb[:, fh * P:(fh + 1) * P],
                            rhs=rhs_sb[:, sl], start=True, stop=True,
                        )
                        # relu + per-partition bias b2[fh*128 + p]
                        nc.scalar.activation(h2sb[:, sl], psW[:], AF.Relu,
                                             bias=b2a[:, fh:fh + 1])
                    # node i's 32 batch-cols: strided slice [s*SB_COLS + i*4 + b]
                    h2.append(h2sb[:].rearrange(
                        "p (s n b) -> p n s b", s=NC_CORES, n=NB_H, b=B_LOC))
                hw_sb = None
                for i in range(NB_H):
                    g = (nb * NB_H + i) // 16          # 16-node headW slab index
                    if i % 16 == 0:
                        hw_sb = sb_hw.tile([P, 32 * HW_P], hd_dt, tag="hwslab")
                        nc.sync.dma_start(
                            hw_sb[:],
                            io["headw_t"][:, g * 32 * HW_P:(g + 1) * 32 * HW_P])
                    for fh in range(2):
                        jj = 2 * (i % 16) + fh
                        last = (nb == nblocks - 1) and (i == NB_H - 1) and (fh == 1)
                        nc.tensor.matmul(
                            ps_hd[:],
                            lhsT=hw_sb[:, jj * HW_P:(jj + 1) * HW_P],
                            rhs=h2[fh][:, i, :, :],
                            start=first_mm,
                            stop=last,
                            skip_group_check=True,
                        )
                        first_mm = False

            part_sb = sb.tile([HW_W, B], f32, tag="part")
            nc.vector.tensor_copy(part_sb[:], ps_hd[0:HW_W, :])
            nc.sync.dma_start(ar_in[:, :], part_sb[:])

            # ================= AllReduce partials =================
            if collectives:
                nc.gpsimd.collective_compute(
                    "AllReduce",
                    mybir.AluOpType.add,
                    replica_groups=[list(range(NC_CORES))],
                    ins=[ar_in[:].opt()],
                    outs=[ar_out[:].opt()],
                )
            else:
                nc.sync.dma_start(ar_out[:, :], ar_in[:, :])
            red_sb = sb.tile([HW_W, B], f32, tag="red")
            nc.sync.dma_start(red_sb[:], ar_out[:, :])

            # ================= final MLP + dueling combine =================
            adv_sb = sb.tile([12, B], f32, tag="adv")
            nc.scalar.activation(adv_sb[:], red_sb[64:76, :], AF.Relu, bias=advb_sb[:])
            # val path
            v1_sb = sb.tile([64, B], f32, tag="v1")
            nc.scalar.activation(v1_sb[:], red_sb[0:64, :], AF.Relu, bias=v1b_sb[:])
            psV = ps_t.tile([64, B], f32, tag="work")
            nc.tensor.matmul(psV[:], lhsT=v2w_sb[:], rhs=v1_sb[:], start=True, stop=True)
            v2_sb = sb.tile([64, B], f32, tag="v2")
            nc.scalar.activation(v2_sb[:], psV[:], AF.Relu, bias=v2b_sb[:])
            psV3 = ps_t.tile([1, B], f32, tag="work")
            nc.tensor.matmul(psV3[:], lhsT=v3w_sb[:], rhs=v2_sb[:], start=True, stop=True)
            val_sb = sb.tile([1, B], f32, tag="val")
            nc.vector.tensor_scalar_add(val_sb[:], psV3[:], v3b_sb[0:1, 0:1])
            # out = cmat.T @ adv + 1.T @ val
            psO = ps_t.tile([12, B], f32, tag="work")
            nc.tensor.matmul(psO[:], lhsT=cmat_sb[:], rhs=adv_sb[:], start=True, stop=False)
            nc.tensor.matmul(psO[:], lhsT=ones1[:, 0:12], rhs=val_sb[:], start=False, stop=True)
            out_sb = sb.tile([12, B], f32, tag="out")
            nc.vector.tensor_copy(out_sb[:], psO[:])
            nc.sync.dma_start(io["out"][:, :], out_sb[:])


# ---------------- driver ----------------

LAST_RESULTS = None

def _input_specs(shared, per_core):
    """name -> (shape, np dtype); per-core entries use per_core[0] shapes."""
    specs = {}
    for k, v in shared.items():
        specs[k] = v
    for k, v in per_core[0].items():
        specs[k] = v
    return specs


def kernel(**inputs) -> np.ndarray:
    import concourse.bacc as bacc
    import concourse.mybir as mybir
    import concourse.tile as tile
    from concourse import bass_utils

    shared, per_core, chunk_tile, nchunk = _prep_host(inputs)

    nc = bacc.Bacc("TRN2", target_bir_lowering=False, debug=False,
                   enable_asserts=False, num_devices=NC_CORES)

    io = {}
    specs = _input_specs(shared, per_core)
    for name, arr in specs.items():
        io[name] = nc.dram_tensor(
            name, list(arr.shape), mybir.dt.from_np(arr.dtype), kind="ExternalInput"
        ).ap()
    io["out"] = nc.dram_tensor(
        "out", [12, B], mybir.dt.float32, kind="ExternalOutput"
    ).ap()

    with tile.TileContext(nc) as tc:
        build_program(nc, tc, chunk_tile, nchunk, io)
    nc.compile()

    in_maps = []
    for j in range(NC_CORES):
        m = dict(shared)
        m.update(per_core[j])
        in_maps.append(m)

    res = bass_utils.run_bass_kernel_spmd(
        nc, in_maps, core_ids=list(range(NC_CORES)),
    )
    global LAST_RESULTS
    LAST_RESULTS = res
    out = res.results[0]["out"]                      # [12, 32]
    return out.T.reshape(B, 3, 4).copy().astype(np.float32)


if __name__ == "__main__":
    rng = np.random.default_rng(0)
    ei = rng.integers(0, N, (2, E)).astype(np.int64)
    demo = {
        "x": rng.standard_normal((B, N, F_IN), np.float32),
        "edge_index": ei,
        "edge_weight": rng.random(E, np.float32),
        "W1": rng.standard_normal((F_IN, F1), np.float32) / 4,
        "b1": np.zeros(F1, np.float32),
        "W2": rng.standard_normal((F1, F2), np.float32) / 11.3,
        "b2": np.zeros(F2, np.float32),
        "advW": rng.standard_normal((N * F2, 12), np.float32) / 1024,
        "advb": np.zeros(12, np.float32),
        "v1W": rng.standard_normal((N * F2, 64), np.float32) / 1024,
        "v1b": np.zeros(64, np.float32),
        "v2W": rng.standard_normal((64, 64), np.float32) / 8,
        "v2b": np.zeros(64, np.float32),
        "v3W": rng.standard_normal((64, 1), np.float32) / 8,
        "v3b": np.zeros(1, np.float32),
    }
    print(kernel(**demo).shape)

